# revision 25
# baseline (speedup 1.0000x reference)
"""KeepTopK kernel for Trainium2.

out[i, j] = x[i, j] if x[i, j] is among the top-8 of row i else 1e6.

Strategy (pure data parallel, 8 cores, 32768 rows each):
  per [128, 4096] block (2048 rows, 16 rows per partition):
    load  : 2MB HWDGE load on the SP ring (nc.sync) — 16KB contiguous
            per partition.  First block loads in eighths so DVE starts
            as early as possible.
    DVE   : per 256-wide row segment: v8 = max8(x_seg)  (the 8 largest,
            so v8[..,8s+7] is that row-segment's top-8 threshold)
    ACT   : per segment: mask = relu(t - x) with the threshold applied
            as a PER-PARTITION bias AP (t = v8[:, 8s+7:8s+8]).  mask is
            +0.0 exactly on every top-8 element (including the 8th,
            since t - t = +0.0, and relu clamps negatives to +0.0) and
            strictly positive elsewhere.
    DVE   : ONE copy_predicated per part, IN PLACE on the x tile:
            copy_predicated(x, mask.bitcast(u32), BETA_const) writes
            BETA wherever mask is nonzero and leaves the top-8 values
            untouched — the x tile becomes the final output with no
            match_replace pass, no Pool pass, and no merge arithmetic.
    store : HWDGE store of the x tile on the ACT ring (nc.scalar).
            Loads and stores ride separate HWDGE rings so neither's
            dependency stalls block the other, and the 16 SDMA engines
            round-robin the rings at packet granularity for the
            balanced ~50/50 load/store HBM split.
  The copy_predicated for part k is emitted after the max8s of part
  k+1 (one-part software stagger) so ACT's mask computation overlaps
  DVE's next max8 batch and DVE never idles.
  Part granularity: eighths at the pipeline ends (short fill/drain
  chains), quarters next to them, halves in steady state.
Output is bit-exact vs the stable top-8 reference except on rows where
the 8th and 9th largest are bit-identical f32 values (threshold keeps
both) — a handful of rows out of 262144 for Gaussian input, far inside
the 2e-2 Frobenius tolerance.
"""
import numpy as np
from contextlib import ExitStack

import concourse.bass as bass
import concourse.mybir as mybir
import concourse.tile as tile
from concourse.bass_utils import run_bass_kernel_spmd

N, E, K = 262144, 256, 8
BETA = 1000000.0
NCORES = 8
ROWS_PER_CORE = N // NCORES          # 32768
ROWS_PER_PART = 16                   # rows packed per SBUF partition
BLOCK_FREE = ROWS_PER_PART * E       # 4096
ROWS_PER_BLOCK = 128 * ROWS_PER_PART  # 2048
NBLOCKS = ROWS_PER_CORE // ROWS_PER_BLOCK  # 16
HALF = BLOCK_FREE // 2               # 2048

MAX_WAITS = 1


def split_sync_waits(nc, max_waits=MAX_WAITS):
    """walrus codegen rejects instructions with more than one embedded sync
    wait; hoist extras onto same-engine NoOps placed immediately before."""
    spill_id = 0
    for f in nc.m.functions:
        for bb in f.blocks:
            insts = list(bb.instructions)
            new_insts = []
            changed = False
            for inst in insts:
                si = inst.sync_info
                waits = list(si.on_wait) if si and si.on_wait else []
                if len(waits) > max_waits:
                    extra = waits[:-max_waits]
                    si.on_wait = waits[-max_waits:]
                    for j in range(0, len(extra), max_waits):
                        nop = mybir.InstNoOp(
                            name=f"waitspill-{spill_id}", ins=[], outs=[])
                        spill_id += 1
                        nop.engine = inst.engine
                        nop.sync_info = type(si)(
                            on_wait=extra[j:j + max_waits], on_update=[])
                        new_insts.append(nop)
                    changed = True
                new_insts.append(inst)
            if changed:
                bb.instructions = new_insts


def build():
    nc = bass.Bass("TRN2", target_bir_lowering=False, debug=False)
    x = nc.dram_tensor("x", [ROWS_PER_CORE, E], mybir.dt.float32,
                       kind="ExternalInput")
    out = nc.dram_tensor("out", [ROWS_PER_CORE, E], mybir.dt.float32,
                         kind="ExternalOutput")
    xap = x.ap()
    oap = out.ap()
    f32 = mybir.dt.float32
    u32 = mybir.dt.uint32
    relu = mybir.ActivationFunctionType.Relu
    with tile.TileContext(nc) as tc:
        with ExitStack() as ctx:
            cpool = ctx.enter_context(tc.tile_pool(name="beta", bufs=1))
            xpool = ctx.enter_context(tc.tile_pool(name="x", bufs=7))
            mpool = ctx.enter_context(tc.tile_pool(name="m", bufs=4))
            vpool = ctx.enter_context(tc.tile_pool(name="v8", bufs=8))
            beta = cpool.tile([128, BLOCK_FREE], f32)
            nc.gpsimd.memset(beta[:], BETA)
            pending = None  # (xt, dst, h0, pw, mt)

            def flush(pend):
                xt_, dst_, h0_, pw_, mt_ = pend
                nc.vector.copy_predicated(
                    xt_[:, h0_:h0_ + pw_], mt_[:].bitcast(u32),
                    beta[:, :pw_])
                nc.scalar.dma_start(dst_[:, h0_:h0_ + pw_],
                                    xt_[:, h0_:h0_ + pw_])

            for b in range(NBLOCKS):
                r0 = b * ROWS_PER_BLOCK
                src = xap[r0:r0 + ROWS_PER_BLOCK, :].rearrange(
                    "(p r) e -> p (r e)", p=128)
                dst = oap[r0:r0 + ROWS_PER_BLOCK, :].rearrange(
                    "(p r) e -> p (r e)", p=128)
                xt = xpool.tile([128, BLOCK_FREE], f32)
                if b == 0:
                    # 4 eighth loads then one half: DVE starts ~1us after
                    # the first eighth lands, while using 3 fewer issue
                    # slots on the SP sequencer so block 1's loads issue
                    # earlier (each issue is ~0.65us; per-chunk receipt
                    # latency makes many small chunks land no earlier)
                    q = BLOCK_FREE // 8
                    for i in range(4):
                        nc.sync.dma_start(xt[:, i * q:(i + 1) * q],
                                          src[:, i * q:(i + 1) * q])
                    nc.sync.dma_start(xt[:, HALF:], src[:, HALF:])
                elif b == 1:
                    nc.sync.dma_start(xt[:, :HALF], src[:, :HALF])
                    nc.sync.dma_start(xt[:, HALF:], src[:, HALF:])
                else:
                    nc.sync.dma_start(xt[:], src)
                # part granularity: eighths at the pipeline ends (shorter
                # fill and drain chains), quarters next to them, halves
                # in steady state
                if b in (0, NBLOCKS - 1):
                    nparts = 8
                elif b in (1, NBLOCKS - 2):
                    nparts = 4
                else:
                    nparts = 2
                pw = BLOCK_FREE // nparts        # columns per part
                segs = pw // E                   # segments per part
                # middle blocks: one whole-block mask tile and ONE
                # copy_predicated + store per block — halves the DVE
                # cross-engine waits and instruction count there
                fuse = nparts == 2
                mt_blk = None
                if fuse:
                    mt_blk = mpool.tile([128, BLOCK_FREE], f32, tag="m")
                for h in range(nparts):
                    h0 = h * pw
                    v8 = vpool.tile([128, 8 * segs], f32, tag="v8")
                    for s in range(segs):
                        seg = slice(h0 + s * E, h0 + (s + 1) * E)
                        nc.vector.max(v8[:, s * 8:(s + 1) * 8], xt[:, seg])
                    if fuse:
                        for s in range(segs):
                            seg = slice(h0 + s * E, h0 + (s + 1) * E)
                            nc.scalar.activation(
                                mt_blk[:, h0 + s * E:h0 + (s + 1) * E],
                                xt[:, seg], relu,
                                bias=v8[:, s * 8 + 7:s * 8 + 8], scale=-1.0)
                        if h == nparts - 1:
                            if pending is not None:
                                flush(pending)
                            pending = (xt, dst, 0, BLOCK_FREE, mt_blk)
                    else:
                        mt = mpool.tile([128, pw], f32, tag="m")
                        for s in range(segs):
                            seg = slice(h0 + s * E, h0 + (s + 1) * E)
                            nc.scalar.activation(
                                mt[:, s * E:(s + 1) * E], xt[:, seg], relu,
                                bias=v8[:, s * 8 + 7:s * 8 + 8], scale=-1.0)
                        if pending is not None:
                            flush(pending)
                        pending = (xt, dst, h0, pw, mt)
            flush(pending)
    split_sync_waits(nc)
    return nc


_nc_cache = None


def _get_nc():
    global _nc_cache
    if _nc_cache is None:
        _nc_cache = build()
    return _nc_cache


def kernel(x: np.ndarray, _trace: bool = False, **_trace_kwargs):
    x = np.ascontiguousarray(np.asarray(x, dtype=np.float32))
    assert x.shape == (N, E), x.shape
    nc = _get_nc()
    in_maps = [
        {"x": x[c * ROWS_PER_CORE:(c + 1) * ROWS_PER_CORE]}
        for c in range(NCORES)
    ]
    res = run_bass_kernel_spmd(nc, in_maps, core_ids=list(range(NCORES)),
                               trace=_trace, **_trace_kwargs)
    out = np.concatenate([res.results[c]["out"] for c in range(NCORES)],
                         axis=0)
    if _trace:
        return out, res
    return out


# revision 26
# speedup vs baseline: 1.0814x; 1.0814x over previous
"""KeepTopK kernel for Trainium2.

out[i, j] = x[i, j] if x[i, j] is among the top-8 of row i else 1e6.

Strategy (pure data parallel, 8 cores, 32768 rows each):
  per [128, 4096] block (2048 rows, 16 rows per partition):
    load  : 2MB HWDGE load on the SP ring (nc.sync) — 16KB contiguous
            per partition.  First block loads in eighths so DVE starts
            as early as possible.
    DVE   : per 256-wide row segment: v8 = max8(x_seg)  (the 8 largest,
            so v8[..,8s+7] is that row-segment's top-8 threshold)
    ACT   : per segment: mask = relu(t - x) with the threshold applied
            as a PER-PARTITION bias AP (t = v8[:, 8s+7:8s+8]).  mask is
            +0.0 exactly on every top-8 element (including the 8th,
            since t - t = +0.0, and relu clamps negatives to +0.0) and
            strictly positive elsewhere.
    DVE   : ONE copy_predicated per part, IN PLACE on the x tile:
            copy_predicated(x, mask.bitcast(u32), BETA_const) writes
            BETA wherever mask is nonzero and leaves the top-8 values
            untouched — the x tile becomes the final output with no
            match_replace pass, no Pool pass, and no merge arithmetic.
    store : HWDGE store of the x tile on the ACT ring (nc.scalar).
            Loads and stores ride separate HWDGE rings so neither's
            dependency stalls block the other, and the 16 SDMA engines
            round-robin the rings at packet granularity for the
            balanced ~50/50 load/store HBM split.
  The copy_predicated for part k is emitted after the max8s of part
  k+1 (one-part software stagger) so ACT's mask computation overlaps
  DVE's next max8 batch and DVE never idles.
  Part granularity: eighths at the pipeline ends (short fill/drain
  chains), quarters next to them, halves in steady state.
Output is bit-exact vs the stable top-8 reference except on rows where
the 8th and 9th largest are bit-identical f32 values (threshold keeps
both) — a handful of rows out of 262144 for Gaussian input, far inside
the 2e-2 Frobenius tolerance.
"""
import numpy as np
from contextlib import ExitStack

import concourse.bass as bass
import concourse.mybir as mybir
import concourse.tile as tile
from concourse.bass_utils import run_bass_kernel_spmd

N, E, K = 262144, 256, 8
BETA = 1000000.0
NCORES = 8
ROWS_PER_CORE = N // NCORES          # 32768
ROWS_PER_PART = 16                   # rows packed per SBUF partition
BLOCK_FREE = ROWS_PER_PART * E       # 4096
ROWS_PER_BLOCK = 128 * ROWS_PER_PART  # 2048
NBLOCKS = ROWS_PER_CORE // ROWS_PER_BLOCK  # 16
HALF = BLOCK_FREE // 2               # 2048

MAX_WAITS = 1


def split_sync_waits(nc, max_waits=MAX_WAITS):
    """walrus codegen rejects instructions with more than one embedded sync
    wait; hoist extras onto same-engine NoOps placed immediately before."""
    spill_id = 0
    for f in nc.m.functions:
        for bb in f.blocks:
            insts = list(bb.instructions)
            new_insts = []
            changed = False
            for inst in insts:
                si = inst.sync_info
                waits = list(si.on_wait) if si and si.on_wait else []
                if len(waits) > max_waits:
                    extra = waits[:-max_waits]
                    si.on_wait = waits[-max_waits:]
                    for j in range(0, len(extra), max_waits):
                        nop = mybir.InstNoOp(
                            name=f"waitspill-{spill_id}", ins=[], outs=[])
                        spill_id += 1
                        nop.engine = inst.engine
                        nop.sync_info = type(si)(
                            on_wait=extra[j:j + max_waits], on_update=[])
                        new_insts.append(nop)
                    changed = True
                new_insts.append(inst)
            if changed:
                bb.instructions = new_insts


def build():
    nc = bass.Bass("TRN2", target_bir_lowering=False, debug=False)
    x = nc.dram_tensor("x", [ROWS_PER_CORE, E], mybir.dt.float32,
                       kind="ExternalInput")
    out = nc.dram_tensor("out", [ROWS_PER_CORE, E], mybir.dt.float32,
                         kind="ExternalOutput")
    xap = x.ap()
    oap = out.ap()
    f32 = mybir.dt.float32
    u32 = mybir.dt.uint32
    relu = mybir.ActivationFunctionType.Relu
    with tile.TileContext(nc) as tc:
        with ExitStack() as ctx:
            cpool = ctx.enter_context(tc.tile_pool(name="beta", bufs=1))
            xpool = ctx.enter_context(tc.tile_pool(name="x", bufs=7))
            mpool = ctx.enter_context(tc.tile_pool(name="m", bufs=4))
            vpool = ctx.enter_context(tc.tile_pool(name="v8", bufs=8))
            beta = cpool.tile([128, BLOCK_FREE], f32)
            nc.gpsimd.memset(beta[:], BETA)
            pending = None  # (xt, dst, h0, pw, mt)

            def flush(pend):
                xt_, dst_, h0_, pw_, mt_ = pend
                nc.vector.copy_predicated(
                    xt_[:, h0_:h0_ + pw_], mt_[:].bitcast(u32),
                    beta[:, :pw_])
                nc.scalar.dma_start(dst_[:, h0_:h0_ + pw_],
                                    xt_[:, h0_:h0_ + pw_])

            for b in range(NBLOCKS):
                r0 = b * ROWS_PER_BLOCK
                src = xap[r0:r0 + ROWS_PER_BLOCK, :].rearrange(
                    "(p r) e -> p (r e)", p=128)
                dst = oap[r0:r0 + ROWS_PER_BLOCK, :].rearrange(
                    "(p r) e -> p (r e)", p=128)
                xt = xpool.tile([128, BLOCK_FREE], f32)
                if b == 0:
                    # 4 eighth loads then one half: DVE starts ~1us after
                    # the first eighth lands, while using 3 fewer issue
                    # slots on the SP sequencer so block 1's loads issue
                    # earlier (each issue is ~0.65us; per-chunk receipt
                    # latency makes many small chunks land no earlier)
                    q = BLOCK_FREE // 8
                    for i in range(4):
                        nc.sync.dma_start(xt[:, i * q:(i + 1) * q],
                                          src[:, i * q:(i + 1) * q])
                    nc.sync.dma_start(xt[:, HALF:], src[:, HALF:])
                elif b == 1:
                    nc.sync.dma_start(xt[:, :HALF], src[:, :HALF])
                    nc.sync.dma_start(xt[:, HALF:], src[:, HALF:])
                else:
                    nc.sync.dma_start(xt[:], src)
                # part granularity: eighths at the pipeline ends (shorter
                # fill and drain chains), quarters next to them, halves
                # in steady state
                if b in (0, NBLOCKS - 1):
                    nparts = 8
                elif b in (1, NBLOCKS - 2):
                    nparts = 4
                else:
                    nparts = 1
                pw = BLOCK_FREE // nparts        # columns per part
                segs = pw // E                   # segments per part
                # middle blocks: one whole-block mask tile and ONE
                # copy_predicated + store per block — halves the DVE
                # cross-engine waits and instruction count there
                fuse = nparts == 1
                mt_blk = None
                if fuse:
                    mt_blk = mpool.tile([128, BLOCK_FREE], f32, tag="m")
                for h in range(nparts):
                    h0 = h * pw
                    v8 = vpool.tile([128, 8 * segs], f32, tag="v8")
                    for s in range(segs):
                        seg = slice(h0 + s * E, h0 + (s + 1) * E)
                        nc.vector.max(v8[:, s * 8:(s + 1) * 8], xt[:, seg])
                    if fuse:
                        for s in range(segs):
                            seg = slice(h0 + s * E, h0 + (s + 1) * E)
                            nc.scalar.activation(
                                mt_blk[:, h0 + s * E:h0 + (s + 1) * E],
                                xt[:, seg], relu,
                                bias=v8[:, s * 8 + 7:s * 8 + 8], scale=-1.0)
                        if h == nparts - 1:
                            if pending is not None:
                                flush(pending)
                            pending = (xt, dst, 0, BLOCK_FREE, mt_blk)
                    else:
                        mt = mpool.tile([128, pw], f32, tag="m")
                        for s in range(segs):
                            seg = slice(h0 + s * E, h0 + (s + 1) * E)
                            nc.scalar.activation(
                                mt[:, s * E:(s + 1) * E], xt[:, seg], relu,
                                bias=v8[:, s * 8 + 7:s * 8 + 8], scale=-1.0)
                        if pending is not None:
                            flush(pending)
                        pending = (xt, dst, h0, pw, mt)
            flush(pending)
    split_sync_waits(nc)
    return nc


_nc_cache = None


def _get_nc():
    global _nc_cache
    if _nc_cache is None:
        _nc_cache = build()
    return _nc_cache


def kernel(x: np.ndarray, _trace: bool = False, **_trace_kwargs):
    x = np.ascontiguousarray(np.asarray(x, dtype=np.float32))
    assert x.shape == (N, E), x.shape
    nc = _get_nc()
    in_maps = [
        {"x": x[c * ROWS_PER_CORE:(c + 1) * ROWS_PER_CORE]}
        for c in range(NCORES)
    ]
    res = run_bass_kernel_spmd(nc, in_maps, core_ids=list(range(NCORES)),
                               trace=_trace, **_trace_kwargs)
    out = np.concatenate([res.results[c]["out"] for c in range(NCORES)],
                         axis=0)
    if _trace:
        return out, res
    return out


# revision 27
# speedup vs baseline: 1.1149x; 1.0310x over previous
"""KeepTopK kernel for Trainium2.

out[i, j] = x[i, j] if x[i, j] is among the top-8 of row i else 1e6.

Strategy (pure data parallel, 8 cores, 32768 rows each):
  per [128, 4096] block (2048 rows, 16 rows per partition):
    load  : 2MB HWDGE load on the SP ring (nc.sync) — 16KB contiguous
            per partition.  First block loads in eighths so DVE starts
            as early as possible.
    DVE   : per 256-wide row segment: v8 = max8(x_seg)  (the 8 largest,
            so v8[..,8s+7] is that row-segment's top-8 threshold)
    ACT   : per segment: mask = relu(t - x) with the threshold applied
            as a PER-PARTITION bias AP (t = v8[:, 8s+7:8s+8]).  mask is
            +0.0 exactly on every top-8 element (including the 8th,
            since t - t = +0.0, and relu clamps negatives to +0.0) and
            strictly positive elsewhere.
    DVE   : ONE copy_predicated per part, IN PLACE on the x tile:
            copy_predicated(x, mask.bitcast(u32), BETA_const) writes
            BETA wherever mask is nonzero and leaves the top-8 values
            untouched — the x tile becomes the final output with no
            match_replace pass, no Pool pass, and no merge arithmetic.
    store : HWDGE store of the x tile on the ACT ring (nc.scalar).
            Loads and stores ride separate HWDGE rings so neither's
            dependency stalls block the other, and the 16 SDMA engines
            round-robin the rings at packet granularity for the
            balanced ~50/50 load/store HBM split.
  The copy_predicated for part k is emitted after the max8s of part
  k+1 (one-part software stagger) so ACT's mask computation overlaps
  DVE's next max8 batch and DVE never idles.
  Part granularity: eighths at the pipeline ends (short fill/drain
  chains), quarters next to them, halves in steady state.
Output is bit-exact vs the stable top-8 reference except on rows where
the 8th and 9th largest are bit-identical f32 values (threshold keeps
both) — a handful of rows out of 262144 for Gaussian input, far inside
the 2e-2 Frobenius tolerance.
"""
import numpy as np
from contextlib import ExitStack

import concourse.bass as bass
import concourse.mybir as mybir
import concourse.tile as tile
from concourse.bass_utils import run_bass_kernel_spmd

N, E, K = 262144, 256, 8
BETA = 1000000.0
NCORES = 8
ROWS_PER_CORE = N // NCORES          # 32768
ROWS_PER_PART = 16                   # rows packed per SBUF partition
BLOCK_FREE = ROWS_PER_PART * E       # 4096
ROWS_PER_BLOCK = 128 * ROWS_PER_PART  # 2048
NBLOCKS = ROWS_PER_CORE // ROWS_PER_BLOCK  # 16
HALF = BLOCK_FREE // 2               # 2048

MAX_WAITS = 1


def split_sync_waits(nc, max_waits=MAX_WAITS):
    """walrus codegen rejects instructions with more than one embedded sync
    wait; hoist extras onto same-engine NoOps placed immediately before."""
    spill_id = 0
    for f in nc.m.functions:
        for bb in f.blocks:
            insts = list(bb.instructions)
            new_insts = []
            changed = False
            for inst in insts:
                si = inst.sync_info
                waits = list(si.on_wait) if si and si.on_wait else []
                if len(waits) > max_waits:
                    extra = waits[:-max_waits]
                    si.on_wait = waits[-max_waits:]
                    for j in range(0, len(extra), max_waits):
                        nop = mybir.InstNoOp(
                            name=f"waitspill-{spill_id}", ins=[], outs=[])
                        spill_id += 1
                        nop.engine = inst.engine
                        nop.sync_info = type(si)(
                            on_wait=extra[j:j + max_waits], on_update=[])
                        new_insts.append(nop)
                    changed = True
                new_insts.append(inst)
            if changed:
                bb.instructions = new_insts


def build():
    nc = bass.Bass("TRN2", target_bir_lowering=False, debug=False)
    x = nc.dram_tensor("x", [ROWS_PER_CORE, E], mybir.dt.float32,
                       kind="ExternalInput")
    out = nc.dram_tensor("out", [ROWS_PER_CORE, E], mybir.dt.float32,
                         kind="ExternalOutput")
    xap = x.ap()
    oap = out.ap()
    f32 = mybir.dt.float32
    u32 = mybir.dt.uint32
    relu = mybir.ActivationFunctionType.Relu
    with tile.TileContext(nc) as tc:
        with ExitStack() as ctx:
            cpool = ctx.enter_context(tc.tile_pool(name="beta", bufs=1))
            xpool = ctx.enter_context(tc.tile_pool(name="x", bufs=7))
            mpool = ctx.enter_context(tc.tile_pool(name="m", bufs=4))
            vpool = ctx.enter_context(tc.tile_pool(name="v8", bufs=8))
            beta = cpool.tile([128, BLOCK_FREE], f32)
            nc.gpsimd.memset(beta[:], BETA)
            pending = None  # (xt, dst, h0, pw, mt)

            def flush(pend):
                xt_, dst_, h0_, pw_, mt_ = pend
                nc.vector.copy_predicated(
                    xt_[:, h0_:h0_ + pw_], mt_[:].bitcast(u32),
                    beta[:, :pw_])
                nc.scalar.dma_start(dst_[:, h0_:h0_ + pw_],
                                    xt_[:, h0_:h0_ + pw_])

            for b in range(NBLOCKS):
                r0 = b * ROWS_PER_BLOCK
                src = xap[r0:r0 + ROWS_PER_BLOCK, :].rearrange(
                    "(p r) e -> p (r e)", p=128)
                dst = oap[r0:r0 + ROWS_PER_BLOCK, :].rearrange(
                    "(p r) e -> p (r e)", p=128)
                xt = xpool.tile([128, BLOCK_FREE], f32)
                if b == 0:
                    # 4 eighth loads then one half: DVE starts ~1us after
                    # the first eighth lands, while using 3 fewer issue
                    # slots on the SP sequencer so block 1's loads issue
                    # earlier (each issue is ~0.65us; per-chunk receipt
                    # latency makes many small chunks land no earlier)
                    q = BLOCK_FREE // 8
                    for i in range(4):
                        nc.sync.dma_start(xt[:, i * q:(i + 1) * q],
                                          src[:, i * q:(i + 1) * q])
                    nc.sync.dma_start(xt[:, HALF:], src[:, HALF:])
                elif b == 1:
                    nc.sync.dma_start(xt[:, :HALF], src[:, :HALF])
                    nc.sync.dma_start(xt[:, HALF:], src[:, HALF:])
                else:
                    nc.sync.dma_start(xt[:], src)
                # part granularity: eighths at the pipeline ends (shorter
                # fill and drain chains), quarters next to them, halves
                # in steady state
                if b in (0, NBLOCKS - 1):
                    nparts = 8
                elif b in (1, NBLOCKS - 2):
                    nparts = 4
                else:
                    nparts = 2
                pw = BLOCK_FREE // nparts        # columns per part
                segs = pw // E                   # segments per part
                # middle blocks: one whole-block mask tile and ONE
                # copy_predicated + store per block — halves the DVE
                # cross-engine waits and instruction count there
                fuse = nparts == 2
                mt_blk = None
                if fuse:
                    mt_blk = mpool.tile([128, BLOCK_FREE], f32, tag="m")
                for h in range(nparts):
                    h0 = h * pw
                    v8 = vpool.tile([128, 8 * segs], f32, tag="v8")
                    for s in range(segs):
                        seg = slice(h0 + s * E, h0 + (s + 1) * E)
                        nc.vector.max(v8[:, s * 8:(s + 1) * 8], xt[:, seg])
                    if fuse:
                        for s in range(segs):
                            seg = slice(h0 + s * E, h0 + (s + 1) * E)
                            nc.scalar.activation(
                                mt_blk[:, h0 + s * E:h0 + (s + 1) * E],
                                xt[:, seg], relu,
                                bias=v8[:, s * 8 + 7:s * 8 + 8], scale=-1.0)
                        if h == nparts - 1:
                            if pending is not None:
                                flush(pending)
                            pending = (xt, dst, 0, BLOCK_FREE, mt_blk)
                    else:
                        mt = mpool.tile([128, pw], f32, tag="m")
                        for s in range(segs):
                            seg = slice(h0 + s * E, h0 + (s + 1) * E)
                            nc.scalar.activation(
                                mt[:, s * E:(s + 1) * E], xt[:, seg], relu,
                                bias=v8[:, s * 8 + 7:s * 8 + 8], scale=-1.0)
                        if pending is not None:
                            flush(pending)
                        pending = (xt, dst, h0, pw, mt)
            flush(pending)
    split_sync_waits(nc)
    return nc


_nc_cache = None


def _get_nc():
    global _nc_cache
    if _nc_cache is None:
        _nc_cache = build()
    return _nc_cache


def kernel(x: np.ndarray, _trace: bool = False, **_trace_kwargs):
    x = np.ascontiguousarray(np.asarray(x, dtype=np.float32))
    assert x.shape == (N, E), x.shape
    nc = _get_nc()
    in_maps = [
        {"x": x[c * ROWS_PER_CORE:(c + 1) * ROWS_PER_CORE]}
        for c in range(NCORES)
    ]
    res = run_bass_kernel_spmd(nc, in_maps, core_ids=list(range(NCORES)),
                               trace=_trace, **_trace_kwargs)
    out = np.concatenate([res.results[c]["out"] for c in range(NCORES)],
                         axis=0)
    if _trace:
        return out, res
    return out
